# revision 2
# baseline (speedup 1.0000x reference)
"""AlphaFold-style gated MSA-row attention on 8 Trainium2 NeuronCores, v2.

Shapes: q_data/kv_data [1,128,256,256], bias [1,128,8,256,256],
nonbatched_bias [1,8,256,256]; heads=8, c=32, out=256.

Pure data-parallel over b2 (16 rows/core). All HBM traffic is bf16
(halves DMA vs f32 baseline). Per row, in transposed activation layout:
  qT/kT [hc, l]   via PE proj (weights stationary), Pool casts to bf16
  skp   [c-pad 128, h, lk]  zero-padded per-head kT built by SBUF->SBUF
        scatter DMAs (gpsimd-issued) into persistent-zero tiles
  L^T   [lk, lq] = per-head K=128 padded matmuls + ONE bf16 identity
        matmul adding bias^T (nb is NOT added on PE)
  P     = exp(L^T + bias^T)  (ACT, PSUM->SBUF bf16)
  T     = P * exp(nb^T)      (DVE bf16 mult; exp(nb^T) precomputed once)
  waT   [lq, h, 33] = T-chunks as STATIONARY weights x [v | 2*ones]
        moving: fuses weighted-avg AND softmax denominator (col 32)
        in one pass, output lands in natural [lq, .] orientation
  merge m = waT * (1+tanh((g+bg)/2)) * recip(2S)  (dense 128-partition
        DVE/Pool ops; gate projected in natural [lq, hc] orientation,
        bg/bo added via K=1 ones-row matmuls)
  mt^T  via 4 PE transpose ops (is_transpose matmul, bf16)
  out   = mt^T Wo^T + bo  -> bf16 DMA out (host casts to f32)
"""

import numpy as np

B1, B2, LQ, LK = 1, 128, 256, 256
QD = KVD = 256
H, C = 8, 32
HC = H * C          # 256
OD = 256
NCORES = 8
RPC = B2 // NCORES  # 16 rows per core

_CACHE = {}


def _build_nc():
    import concourse.bass as bass
    import concourse.bacc as bacc
    import concourse.mybir as mybir
    from concourse.tile import TileContext
    from concourse.bass import ts
    from concourse.tile_rust import add_dep_helper

    f32 = mybir.dt.float32
    bf16 = mybir.dt.bfloat16
    AF = mybir.ActivationFunctionType
    ALU = mybir.AluOpType

    nc = bacc.Bacc()

    qdT_d = nc.declare_dram_parameter("qdT", [RPC // 2, 2, 128, 2, LQ], bf16, isOutput=False)
    kvdT_d = nc.declare_dram_parameter("kvdT", [RPC // 2, 2, 128, 2, LK], bf16, isOutput=False)
    biasT_d = nc.declare_dram_parameter("biasT", [RPC, 2, 128, H, LQ], bf16, isOutput=False)
    nbT_d = nc.declare_dram_parameter("nbT", [2, 128, H, LQ], bf16, isOutput=False)
    wqT_d = nc.declare_dram_parameter("wqT", [QD, HC], bf16, isOutput=False)
    wkT_d = nc.declare_dram_parameter("wkT", [KVD, HC], bf16, isOutput=False)
    wvT_d = nc.declare_dram_parameter("wvT", [KVD, HC], bf16, isOutput=False)
    wgT_d = nc.declare_dram_parameter("wgT", [QD, HC], bf16, isOutput=False)
    woT_d = nc.declare_dram_parameter("woT", [HC, OD], bf16, isOutput=False)
    ident_d = nc.declare_dram_parameter("ident", [128, 128], bf16, isOutput=False)
    onesr_d = nc.declare_dram_parameter("onesr", [1, 128], bf16, isOutput=False)
    bgr_d = nc.declare_dram_parameter("bgr", [1, HC], bf16, isOutput=False)
    bor_d = nc.declare_dram_parameter("bor", [1, OD], bf16, isOutput=False)
    out_d = nc.declare_dram_parameter("out", [RPC, 2, 128, OD], bf16, isOutput=True)

    def chain(mms):
        for a, b in zip(mms, mms[1:]):
            add_dep_helper(b.ins, a.ins, sync=False, reason="psum bank group order")

    with TileContext(nc) as tc:
        with tc.tile_pool(name="const", bufs=1) as constp, \
             tc.tile_pool(name="io", bufs=2) as iop, \
             tc.tile_pool(name="act", bufs=2) as actp, \
             tc.tile_pool(name="ps", bufs=1, space="PSUM") as psp:

            # ---- one-time loads ----
            wq = constp.tile([128, 2, HC], bf16, name="wq")
            wk = constp.tile([128, 2, HC], bf16, name="wk")
            wv = constp.tile([128, 2, HC], bf16, name="wv")
            wg = constp.tile([128, 2, HC], bf16, name="wg")
            wo = constp.tile([128, 2, OD], bf16, name="wo")
            for t, d in ((wq, wqT_d), (wk, wkT_d), (wv, wvT_d), (wg, wgT_d)):
                nc.sync.dma_start(out=t, in_=d.rearrange("(c p) h -> p c h", p=128))
            nbt = constp.tile([128, 2, H, LQ], bf16, name="nbt")
            enb = constp.tile([128, 2, H, LQ], bf16, name="enb")
            ident = constp.tile([128, 128], bf16, name="ident")
            onesr = constp.tile([1, 128], bf16, name="onesr")
            bgr = constp.tile([1, HC], bf16, name="bgr")
            bor = constp.tile([1, OD], bf16, name="bor")

            def load_consts():
                nc.sync.dma_start(out=wo, in_=woT_d.rearrange("(c p) h -> p c h", p=128))
                nc.sync.dma_start(out=nbt, in_=nbT_d.rearrange("c p h l -> p c h l"))
                nc.sync.dma_start(out=ident, in_=ident_d[:, :])
                nc.sync.dma_start(out=onesr, in_=onesr_d[:, :])
                nc.sync.dma_start(out=bgr, in_=bgr_d[:, :])
                nc.sync.dma_start(out=bor, in_=bor_d[:, :])
                for lkc in range(2):
                    nc.scalar.activation(enb[:, lkc, :, :], nbt[:, lkc, :, :], AF.Exp)

            # persistent zero-padded per-head kT (pair-scoped, zeros written once)
            skp = []
            for i in range(2):
                t = constp.tile([128, H, 2, 2, 128], bf16, name=f"skp{i}")
                nc.vector.memset(t, 0.0)
                skp.append(t)

            def emit_pair_front(sp):
                qdt = iop.tile([128, 2, 2, LQ], bf16, tag="qdt", name="qdt")
                kvdt = iop.tile([128, 2, 2, LK], bf16, tag="kvdt", name="kvdt")
                nc.sync.dma_start(out=qdt, in_=qdT_d[sp].rearrange("c p r l -> p c r l"))
                nc.sync.dma_start(out=kvdt, in_=kvdT_d[sp].rearrange("c p r l -> p c r l"))
                return dict(qdt=qdt, kvdt=kvdt)

            def make_pair_closures(sp, pf):
                """Pair-scope work as single-mm closures (casts/pads attached)."""
                qdt, kvdt = pf["qdt"], pf["kvdt"]
                sq = actp.tile([128, 2, 2, LQ], bf16, tag="sq", name="sq")
                sk = actp.tile([128, 2, 2, LK], bf16, tag="sk", name="sk")
                skp_s = skp[sp % 2]
                st = dict(sp=sp, sq=sq, sk=sk, skp=skp_s,
                          tv_r=[None, None], sv_r=[None, None])
                cls = []
                boxes = {}

                def mk_qk(which, j, c2):
                    def f():
                        key = (which, j)
                        if c2 == 0:
                            boxes[key] = dict(
                                p=psp.tile([128, 2, LQ], f32, tag="proj", bufs=2,
                                           name=f"p{which}{j}"), mms=[])
                        box = boxes[key]
                        w, rhs_t = (wq, qdt) if which == "q" else (wk, kvdt)
                        box["mms"].append(nc.tensor.matmul(
                            box["p"][:, :, :], w[:, c2, ts(j, 128)], rhs_t[:, c2, :, :],
                            start=(c2 == 0), stop=(c2 == 1)))
                        if c2 == 1:
                            chain(box["mms"])
                            if which == "q":
                                nc.scalar.copy(sq[:, j, :, :], box["p"])
                            else:
                                nc.vector.tensor_copy(sk[:, j, :, :], box["p"])
                                if j == 1:
                                    for h in range(H):
                                        b0 = 32 * (h % 4)
                                        nc.gpsimd.dma_start(
                                            out=skp_s[b0:b0 + 32, h, :, :, :],
                                            in_=sk[b0:b0 + 32, h // 4, :, :].rearrange(
                                                "p r (k l) -> p r k l", k=2))
                    return f

                for which in ("q", "k"):
                    for j in range(2):
                        for c2 in range(2):
                            cls.append(mk_qk(which, j, c2))
                if sp == 0:
                    cls.append(load_consts)

                def mk_pg(rr, i):
                    # i 0..5: (lqc0: c2=0, c2=1, K1-bg), (lqc1: ...)
                    def f():
                        key = ("g", rr)
                        if i == 0:
                            boxes[key] = dict(
                                p=psp.tile([128, 2, LQ], f32, tag="proj", bufs=2,
                                           name="pg"), mms=[])
                        box = boxes[key]
                        pg = box["p"]
                        lqc, k = divmod(i, 3)
                        if k < 2:
                            box["mms"].append(nc.tensor.matmul(
                                pg[:, lqc, :], qdt[:, k, rr, ts(lqc, 128)], wg[:, k, :],
                                start=(i == 0), stop=False, skip_group_check=True))
                        else:
                            box["mms"].append(nc.tensor.matmul(
                                pg[:, lqc, :], onesr, bgr,
                                start=False, stop=(i == 5), skip_group_check=True))
                            if i == 5:
                                chain(box["mms"])
                                tv = actp.tile([128, 2, LQ], bf16, tag="tv", bufs=3,
                                               name="tv")
                                nc.scalar.activation(tv, pg, AF.Tanh, scale=0.5)
                                st["tv_r"][rr] = tv
                    return f

                def mk_pv(rr, i):
                    # i 0..3: (lkc, c2)
                    def f():
                        key = ("v", rr)
                        if i == 0:
                            boxes[key] = dict(
                                p=psp.tile([128, 2, HC], f32, tag="proj", bufs=2,
                                           name="pv"), mms=[])
                        box = boxes[key]
                        pv = box["p"]
                        lkc, c2 = divmod(i, 2)
                        box["mms"].append(nc.tensor.matmul(
                            pv[:, lkc, :], kvdt[:, c2, rr, ts(lkc, 128)], wv[:, c2, :],
                            start=(i == 0), stop=(i == 3), skip_group_check=True))
                        if i == 3:
                            chain(box["mms"])
                            sv = actp.tile([128, 2, H, 33], bf16, tag="sv", bufs=3,
                                           name="sv")
                            nc.gpsimd.memset(sv[:, :, :, 32], 2.0)
                            for lkc2 in range(2):
                                nc.vector.tensor_copy(
                                    sv[:, lkc2, :, 0:32],
                                    pv[:, lkc2, :].rearrange("p (h c) -> p h c", h=H))
                            st["sv_r"][rr] = sv
                    return f

                for rr in range(2):
                    for i in range(6):
                        cls.append(mk_pg(rr, i))
                    for i in range(4):
                        cls.append(mk_pv(rr, i))
                return st, cls

            def emit_front(s):
                bias_sb = iop.tile([128, 2, H, LQ], bf16, tag="bias", bufs=3, name="bias_sb")
                nc.sync.dma_start(out=bias_sb, in_=biasT_d[s].rearrange("c p h l -> p c h l"))
                return bias_sb

            def make_log_closures(st, rr, bias_sb):
                """Single-mm closures for logits+bias+exp of row s = 2sp+rr."""
                sq, skp_s = st["sq"], st["skp"]
                et = actp.tile([128, 2, H, LQ], bf16, tag="et", name="et")
                tt = actp.tile([128, 2, H, LQ], bf16, tag="T", name="T")
                cls = []
                for lkc in range(2):
                    for g in range(2):
                        box = dict(pL=None, mms=[])

                        def mk(lkc, g, i, box=None):
                            def f():
                                if i == 0:
                                    box["pL"] = psp.tile([128, 4, LQ], f32, tag="L",
                                                         bufs=2, name="pL")
                                pL = box["pL"]
                                if i in (0, 1, 3, 4):
                                    slot = {0: 0, 1: 1, 3: 2, 4: 3}[i]
                                    h = 4 * g + slot
                                    box["mms"].append(nc.tensor.matmul(
                                        pL[:, slot, :], skp_s[:, h, rr, lkc, :],
                                        sq[:, h // 4, rr, :],
                                        start=(slot in (0, 2)), stop=False,
                                        skip_group_check=True))
                                else:
                                    pr = (i == 5)
                                    box["mms"].append(nc.tensor.matmul(
                                        pL[:, 2 * pr:2 * pr + 2, :], ident,
                                        bias_sb[:, lkc, 4 * g + 2 * pr:4 * g + 2 * pr + 2, :],
                                        start=False, stop=True, skip_group_check=True))
                                    if i == 5:
                                        chain(box["mms"])
                                        nc.scalar.activation(
                                            et[:, lkc, ts(g, 4), :], pL, AF.Exp)
                                        if g == 1:
                                            nc.vector.tensor_tensor(
                                                out=tt[:, lkc, :, :], in0=et[:, lkc, :, :],
                                                in1=enb[:, lkc, :, :], op=ALU.mult)
                            return f

                        for i in (0, 1, 2, 3, 4, 5):
                            cls.append(mk(lkc, g, i, box))
                return cls, tt

            def make_att_closures(st, rr, tt):
                """Single-mm closures for EFS/merge/tp/out of row s, ordered so
                each PSUM wt-ring bank alternates A,B and every PE consumer of a
                DVE result has big matmuls interleaved before it."""
                sp = st["sp"]
                s = 2 * sp + rr
                tv, sv = st["tv_r"][rr], st["sv_r"][rr]
                mt = actp.tile([128, 2, HC], bf16, tag="mt", name="mt")
                ob = actp.tile([128, 2, OD], bf16, tag="ob", name="ob")
                small = []
                state = {}

                def mk_efs(lqc, h, lkc):
                    def f():
                        if h == 0 and lkc == 0:
                            state[("wa", lqc)] = dict(
                                t=psp.tile([128, 512], f32, tag="wt", bufs=2,
                                           name="wt"), mms=[])
                        box = state[("wa", lqc)]
                        waT = box["t"][:, 0:264].rearrange("p (h x) -> p h x", x=33)
                        box["mms"].append(nc.tensor.matmul(
                            waT[:, h, :], tt[:, lkc, h, ts(lqc, 128)],
                            sv[:, lkc, h, :],
                            start=(h == 0 and lkc == 0), stop=(h == 7 and lkc == 1),
                            skip_group_check=True))
                        if h == 7 and lkc == 1:
                            chain(box["mms"])
                            rs = actp.tile([128, H], f32, tag="rs", bufs=2, name="rs")
                            nc.vector.reciprocal_approx_fast(out=rs, in_=waT[:, :, 32])
                            tmp = actp.tile([128, H, 32], bf16, tag="tmp", bufs=2,
                                            name="tmp")
                            nc.vector.scalar_tensor_tensor(
                                out=tmp,
                                in0=tv[:, lqc, :].rearrange("p (h c) -> p h c", h=H),
                                scalar=1.0, in1=waT[:, :, 0:32],
                                op0=ALU.add, op1=ALU.mult)
                            nc.vector.scalar_tensor_tensor(
                                out=mt[:, lqc, :].rearrange("p (h c) -> p h c", h=H),
                                in0=tmp, scalar=1.0,
                                in1=rs[:, :].unsqueeze(2).broadcast_to([128, H, 32]),
                                op0=ALU.mult, op1=ALU.mult)
                    return f

                def mk_tp(lqc, hcc):
                    def f():
                        if hcc == 0:
                            state[("tp", lqc)] = dict(
                                t=psp.tile([128, 512], f32, tag="wt", bufs=2,
                                           name="wt"), mms=[])
                        box = state[("tp", lqc)]
                        mtT = box["t"][:, 0:128].bitcast(bf16).rearrange(
                            "p (a l) -> p a l", a=2)
                        box["mms"].append(nc.tensor.matmul(
                            mtT[:, hcc, :], mt[:, lqc, ts(hcc, 128)], ident,
                            is_transpose=True, start=(hcc == 0), stop=(hcc == 1),
                            skip_group_check=True))
                        if hcc == 1:
                            chain(box["mms"])
                            mts = actp.tile([128, 2, 128], bf16, tag="mts", bufs=2,
                                            name="mts")
                            nc.vector.tensor_copy(mts, mtT)
                            state[("mts", lqc)] = mts
                    return f

                def mk_po(lqc, k):
                    def f():
                        if k == 0:
                            state[("po", lqc)] = dict(
                                t=psp.tile([128, 512], f32, tag="wt", bufs=2,
                                           name="wt"), mms=[])
                        box = state[("po", lqc)]
                        po = box["t"][:, 0:256]
                        mts = state[("mts", lqc)]
                        if k < 2:
                            box["mms"].append(nc.tensor.matmul(
                                po, mts[:, k, :], wo[:, k, :],
                                start=(k == 0), stop=False, skip_group_check=True))
                        else:
                            box["mms"].append(nc.tensor.matmul(
                                po, onesr, bor,
                                start=False, stop=True, skip_group_check=True))
                            chain(box["mms"])
                            nc.scalar.copy(ob[:, lqc, :], po)
                            nc.sync.dma_start(
                                out=out_d[s, lqc].rearrange("p o -> p o"),
                                in_=ob[:, lqc, :])
                    return f

                # interleave lqc0/lqc1 EFS mms so consecutive matmuls hit
                # alternating PSUM banks (hides same-bank turnaround)
                for h in range(H):
                    for lkc in range(2):
                        for lqc in range(2):
                            small.append(mk_efs(lqc, h, lkc))
                for lqc in range(2):
                    for hcc in range(2):
                        small.append(mk_tp(lqc, hcc))
                    for k in range(3):
                        small.append(mk_po(lqc, k))
                return small

            prev = None
            for sp in range(RPC // 2):
                pf = emit_pair_front(sp)
                st, pair_cls = make_pair_closures(sp, pf)
                for f in pair_cls:
                    f()
                st["bias"] = [emit_front(2 * sp), emit_front(2 * sp + 1)]
                if prev is not None:
                    for rr in range(2):
                        logs, tt = make_log_closures(prev, rr, prev["bias"][rr])
                        for f in logs:
                            f()
                        for f in make_att_closures(prev, rr, tt):
                            f()
                prev = st
            for rr in range(2):
                logs, tt = make_log_closures(prev, rr, prev["bias"][rr])
                for f in logs:
                    f()
                for f in make_att_closures(prev, rr, tt):
                    f()

    nc.compile()
    return nc


def _prep_inputs(q_data, kv_data, bias, nonbatched_bias, Wq, Wk, Wv, Wg, bg, Wo, bo):
    """Host-side data marshalling only (layout + dtype cast; qscale folded
    into Wq as a constant rescale)."""
    import ml_dtypes
    c = np.ascontiguousarray
    bf = ml_dtypes.bfloat16
    f = np.float32

    def pair_layout(x):  # x [b2, l, d] -> [b2/2, 2(c2), 128, 2(row), l]
        xt = np.swapaxes(np.asarray(x, f), 1, 2)          # [b2, d, l]
        xt = xt.reshape(B2 // 2, 2, 2, 128, xt.shape[-1])  # [sp, row, c2, p, l]
        return c(np.transpose(xt, (0, 2, 3, 1, 4))).astype(bf)

    qdT = pair_layout(q_data[0])
    kvdT = pair_layout(kv_data[0])
    biasT = c(np.transpose(np.asarray(bias[0], f), (0, 3, 1, 2))).reshape(B2, 2, 128, H, LQ).astype(bf)
    nbT = c(np.transpose(np.asarray(nonbatched_bias[0], f), (2, 0, 1))).reshape(2, 128, H, LQ).astype(bf)
    qscale = float(C) ** -0.5
    wqT = c((np.asarray(Wq, f) * qscale).T).astype(bf)
    wkT = c(np.asarray(Wk, f).T).astype(bf)
    wvT = c(np.asarray(Wv, f).T).astype(bf)
    wgT = c(np.asarray(Wg, f).T).astype(bf)
    woT = c(np.asarray(Wo, f).T).astype(bf)
    ident = np.eye(128, dtype=f).astype(bf)
    onesr = np.ones((1, 128), f).astype(bf)
    bgr = np.asarray(bg, f)[None, :].astype(bf)
    bor = np.asarray(bo, f)[None, :].astype(bf)

    in_maps = []
    for core in range(NCORES):
        sl = slice(core * RPC, (core + 1) * RPC)
        in_maps.append(dict(
            qdT=c(qdT[core * RPC // 2:(core + 1) * RPC // 2]),
            kvdT=c(kvdT[core * RPC // 2:(core + 1) * RPC // 2]),
            biasT=c(biasT[sl]), nbT=nbT,
            wqT=wqT, wkT=wkT, wvT=wvT, wgT=wgT, woT=woT,
            ident=ident, onesr=onesr, bgr=bgr, bor=bor,
        ))
    return in_maps


def kernel(q_data, kv_data, bias, nonbatched_bias, Wq, Wk, Wv, Wg, bg, Wo, bo,
           _trace=False):
    from concourse.bass_utils import run_bass_kernel_spmd

    if "nc" not in _CACHE:
        _CACHE["nc"] = _build_nc()
    nc = _CACHE["nc"]
    in_maps = _prep_inputs(q_data, kv_data, bias, nonbatched_bias,
                           Wq, Wk, Wv, Wg, bg, Wo, bo)
    res = run_bass_kernel_spmd(nc, in_maps, list(range(NCORES)), trace=_trace)
    # out shard [RPC, 2, 128, OD] bf16 -> [RPC, LQ, OD] f32
    out = np.concatenate(
        [np.asarray(res.results[i]["out"]).astype(np.float32).reshape(RPC, LQ, OD)
         for i in range(NCORES)], axis=0)
    out = out.reshape(B1, B2, LQ, OD)
    if _trace:
        _CACHE["last_result"] = res
    return out


# revision 3
# speedup vs baseline: 1.0078x; 1.0078x over previous
"""AlphaFold-style gated MSA-row attention on 8 Trainium2 NeuronCores, v2.

Shapes: q_data/kv_data [1,128,256,256], bias [1,128,8,256,256],
nonbatched_bias [1,8,256,256]; heads=8, c=32, out=256.

Pure data-parallel over b2 (16 rows/core). All HBM traffic is bf16
(halves DMA vs f32 baseline). Per row, in transposed activation layout:
  qT/kT [hc, l]   via PE proj (weights stationary); ACT/DVE casts to bf16
  skp   [c-pad 128, h, row, lk]  zero-padded per-head kT built per PAIR by
        SBUF->SBUF scatter DMAs (gpsimd-issued) into persistent-zero tiles
  L^T   [lk, lq] = per-head K=128 padded matmuls + bf16 identity matmuls
        adding bias^T, 4 heads per 2-bank PSUM tile (nb NOT added on PE)
  P     = exp(L^T + bias^T)  (ACT, one 1024-el act per 2-bank tile)
  T     = P * exp(nb^T)      (DVE bf16 mult; exp(nb^T) precomputed once)
  waT   [lq, h, 33] = T-chunks as STATIONARY weights x [v | 2*ones]
        moving: fuses weighted-avg AND softmax denominator (col 32) in one
        pass; output lands in natural [lq, .] orientation so the merge and
        reciprocal are dense 128-partition ops (no band fragmentation);
        consecutive matmuls alternate the two wt-ring PSUM banks
  merge m = waT * (1+tanh((g+bg)/2)) * recip(2S)  (DVE; rs broadcast via
        stride-0 AP; gate projected in natural [lq, hc] orientation;
        bg/bo added via K=1 ones-row matmuls)
  mt^T  via PE transpose ops (is_transpose matmul, bf16) into the shared
        wt-ring bank, then out = mt^T Wo^T + bo -> bf16 DMA (host -> f32)
  PSUM: proj ring 2 banks, logits ring 2x2-bank tiles, wt ring 2 banks
        shared by waT/mtT/po in strict A,B alternation.
"""

import numpy as np

B1, B2, LQ, LK = 1, 128, 256, 256
QD = KVD = 256
H, C = 8, 32
HC = H * C          # 256
OD = 256
NCORES = 8
RPC = B2 // NCORES  # 16 rows per core

_CACHE = {}


def _build_nc():
    import concourse.bass as bass
    import concourse.bacc as bacc
    import concourse.mybir as mybir
    from concourse.tile import TileContext
    from concourse.bass import ts
    from concourse.tile_rust import add_dep_helper

    f32 = mybir.dt.float32
    bf16 = mybir.dt.bfloat16
    AF = mybir.ActivationFunctionType
    ALU = mybir.AluOpType

    nc = bacc.Bacc()

    qdT_d = nc.declare_dram_parameter("qdT", [RPC // 2, 2, 128, 2, LQ], bf16, isOutput=False)
    kvdT_d = nc.declare_dram_parameter("kvdT", [RPC // 2, 2, 128, 2, LK], bf16, isOutput=False)
    biasT_d = nc.declare_dram_parameter("biasT", [RPC, 2, 128, H, LQ], bf16, isOutput=False)
    nbT_d = nc.declare_dram_parameter("nbT", [2, 128, H, LQ], bf16, isOutput=False)
    wqT_d = nc.declare_dram_parameter("wqT", [QD, HC], bf16, isOutput=False)
    wkT_d = nc.declare_dram_parameter("wkT", [KVD, HC], bf16, isOutput=False)
    wvT_d = nc.declare_dram_parameter("wvT", [KVD, HC], bf16, isOutput=False)
    wgT_d = nc.declare_dram_parameter("wgT", [QD, HC], bf16, isOutput=False)
    woT_d = nc.declare_dram_parameter("woT", [HC, OD], bf16, isOutput=False)
    ident_d = nc.declare_dram_parameter("ident", [128, 128], bf16, isOutput=False)
    onesr_d = nc.declare_dram_parameter("onesr", [1, 128], bf16, isOutput=False)
    bgr_d = nc.declare_dram_parameter("bgr", [1, HC], bf16, isOutput=False)
    bor_d = nc.declare_dram_parameter("bor", [1, OD], bf16, isOutput=False)
    out_d = nc.declare_dram_parameter("out", [RPC, 2, 128, OD], bf16, isOutput=True)

    def chain(mms):
        for a, b in zip(mms, mms[1:]):
            add_dep_helper(b.ins, a.ins, sync=False, reason="psum bank group order")

    with TileContext(nc) as tc:
        with tc.tile_pool(name="const", bufs=1) as constp, \
             tc.tile_pool(name="io", bufs=2) as iop, \
             tc.tile_pool(name="act", bufs=2) as actp, \
             tc.tile_pool(name="ps", bufs=1, space="PSUM") as psp:

            # ---- one-time loads ----
            wq = constp.tile([128, 2, HC], bf16, name="wq")
            wk = constp.tile([128, 2, HC], bf16, name="wk")
            wv = constp.tile([128, 2, HC], bf16, name="wv")
            wg = constp.tile([128, 2, HC], bf16, name="wg")
            wo = constp.tile([128, 2, OD], bf16, name="wo")
            for t, d in ((wq, wqT_d), (wk, wkT_d), (wv, wvT_d), (wg, wgT_d)):
                nc.sync.dma_start(out=t, in_=d.rearrange("(c p) h -> p c h", p=128))
            nbt = constp.tile([128, 2, H, LQ], bf16, name="nbt")
            enb = constp.tile([128, 2, H, LQ], bf16, name="enb")
            ident = constp.tile([128, 128], bf16, name="ident")
            onesr = constp.tile([1, 128], bf16, name="onesr")
            bgr = constp.tile([1, HC], bf16, name="bgr")
            bor = constp.tile([1, OD], bf16, name="bor")

            def load_consts():
                nc.sync.dma_start(out=wo, in_=woT_d.rearrange("(c p) h -> p c h", p=128))
                nc.sync.dma_start(out=nbt, in_=nbT_d.rearrange("c p h l -> p c h l"))
                nc.sync.dma_start(out=ident, in_=ident_d[:, :])
                nc.sync.dma_start(out=onesr, in_=onesr_d[:, :])
                nc.sync.dma_start(out=bgr, in_=bgr_d[:, :])
                nc.sync.dma_start(out=bor, in_=bor_d[:, :])
                for lkc in range(2):
                    nc.scalar.activation(enb[:, lkc, :, :], nbt[:, lkc, :, :], AF.Exp)

            # persistent zero-padded per-head kT (pair-scoped, zeros written once)
            skp = []
            for i in range(2):
                t = constp.tile([128, H, 2, 2, 128], bf16, name=f"skp{i}")
                nc.vector.memset(t, 0.0)
                skp.append(t)

            def emit_pair_front(sp):
                qdt = iop.tile([128, 2, 2, LQ], bf16, tag="qdt", name="qdt")
                kvdt = iop.tile([128, 2, 2, LK], bf16, tag="kvdt", name="kvdt")
                nc.sync.dma_start(out=qdt, in_=qdT_d[sp].rearrange("c p r l -> p c r l"))
                nc.sync.dma_start(out=kvdt, in_=kvdT_d[sp].rearrange("c p r l -> p c r l"))
                return dict(qdt=qdt, kvdt=kvdt)

            def make_pair_closures(sp, pf):
                """Pair-scope work as single-mm closures (casts/pads attached)."""
                qdt, kvdt = pf["qdt"], pf["kvdt"]
                sq = actp.tile([128, 2, 2, LQ], bf16, tag="sq", name="sq")
                sk = actp.tile([128, 2, 2, LK], bf16, tag="sk", name="sk")
                skp_s = skp[sp % 2]
                st = dict(sp=sp, sq=sq, sk=sk, skp=skp_s,
                          tv_r=[None, None], sv_r=[None, None])
                cls = []
                boxes = {}

                def mk_qk(which, j, c2):
                    def f():
                        key = (which, j)
                        if c2 == 0:
                            boxes[key] = dict(
                                p=psp.tile([128, 2, LQ], f32, tag="proj", bufs=2,
                                           name=f"p{which}{j}"), mms=[])
                        box = boxes[key]
                        w, rhs_t = (wq, qdt) if which == "q" else (wk, kvdt)
                        box["mms"].append(nc.tensor.matmul(
                            box["p"][:, :, :], w[:, c2, ts(j, 128)], rhs_t[:, c2, :, :],
                            start=(c2 == 0), stop=(c2 == 1)))
                        if c2 == 1:
                            chain(box["mms"])
                            if which == "q":
                                nc.scalar.copy(sq[:, j, :, :], box["p"])
                            else:
                                nc.vector.tensor_copy(sk[:, j, :, :], box["p"])
                                if j == 1:
                                    for h in range(H):
                                        b0 = 32 * (h % 4)
                                        nc.gpsimd.dma_start(
                                            out=skp_s[b0:b0 + 32, h, :, :, :],
                                            in_=sk[b0:b0 + 32, h // 4, :, :].rearrange(
                                                "p r (k l) -> p r k l", k=2))
                    return f

                for which in ("q", "k"):
                    for j in range(2):
                        for c2 in range(2):
                            cls.append(mk_qk(which, j, c2))
                if sp == 0:
                    cls.append(load_consts)

                def mk_pg(rr, i):
                    # i 0..5: (lqc0: c2=0, c2=1, K1-bg), (lqc1: ...)
                    def f():
                        key = ("g", rr)
                        if i == 0:
                            boxes[key] = dict(
                                p=psp.tile([128, 2, LQ], f32, tag="proj", bufs=2,
                                           name="pg"), mms=[])
                        box = boxes[key]
                        pg = box["p"]
                        lqc, k = divmod(i, 3)
                        if k < 2:
                            box["mms"].append(nc.tensor.matmul(
                                pg[:, lqc, :], qdt[:, k, rr, ts(lqc, 128)], wg[:, k, :],
                                start=(i == 0), stop=False, skip_group_check=True))
                        else:
                            box["mms"].append(nc.tensor.matmul(
                                pg[:, lqc, :], onesr, bgr,
                                start=False, stop=(i == 5), skip_group_check=True))
                            if i == 5:
                                chain(box["mms"])
                                tv = actp.tile([128, 2, LQ], bf16, tag="tv", bufs=3,
                                               name="tv")
                                nc.scalar.activation(tv, pg, AF.Tanh, scale=0.5)
                                st["tv_r"][rr] = tv
                    return f

                def mk_pv(rr, i):
                    # i 0..3: (lkc, c2)
                    def f():
                        key = ("v", rr)
                        if i == 0:
                            boxes[key] = dict(
                                p=psp.tile([128, 2, HC], f32, tag="proj", bufs=2,
                                           name="pv"), mms=[])
                        box = boxes[key]
                        pv = box["p"]
                        lkc, c2 = divmod(i, 2)
                        box["mms"].append(nc.tensor.matmul(
                            pv[:, lkc, :], kvdt[:, c2, rr, ts(lkc, 128)], wv[:, c2, :],
                            start=(i == 0), stop=(i == 3), skip_group_check=True))
                        if i == 3:
                            chain(box["mms"])
                            sv = actp.tile([128, 2, H, 33], bf16, tag="sv", bufs=3,
                                           name="sv")
                            nc.gpsimd.memset(sv[:, :, :, 32], 2.0)
                            for lkc2 in range(2):
                                nc.vector.tensor_copy(
                                    sv[:, lkc2, :, 0:32],
                                    pv[:, lkc2, :].rearrange("p (h c) -> p h c", h=H))
                            st["sv_r"][rr] = sv
                    return f

                for rr in range(2):
                    for i in range(6):
                        cls.append(mk_pg(rr, i))
                    for i in range(4):
                        cls.append(mk_pv(rr, i))
                return st, cls

            def emit_front(s):
                bias_sb = iop.tile([128, 2, H, LQ], bf16, tag="bias", bufs=3, name="bias_sb")
                nc.sync.dma_start(out=bias_sb, in_=biasT_d[s].rearrange("c p h l -> p c h l"))
                return bias_sb

            def make_log_closures(st, rr, bias_sb):
                """Single-mm closures for logits+bias+exp of row s = 2sp+rr."""
                sq, skp_s = st["sq"], st["skp"]
                et = actp.tile([128, 2, H, LQ], bf16, tag="et", name="et")
                tt = actp.tile([128, 2, H, LQ], bf16, tag="T", name="T")
                cls = []
                for lkc in range(2):
                    for g in range(2):
                        box = dict(pL=None, mms=[])

                        def mk(lkc, g, i, box=None):
                            def f():
                                if i == 0:
                                    box["pL"] = psp.tile([128, 4, LQ], f32, tag="L",
                                                         bufs=2, name="pL")
                                pL = box["pL"]
                                if i in (0, 1, 3, 4):
                                    slot = {0: 0, 1: 1, 3: 2, 4: 3}[i]
                                    h = 4 * g + slot
                                    box["mms"].append(nc.tensor.matmul(
                                        pL[:, slot, :], skp_s[:, h, rr, lkc, :],
                                        sq[:, h // 4, rr, :],
                                        start=(slot in (0, 2)), stop=False,
                                        skip_group_check=True))
                                else:
                                    pr = (i == 5)
                                    box["mms"].append(nc.tensor.matmul(
                                        pL[:, 2 * pr:2 * pr + 2, :], ident,
                                        bias_sb[:, lkc, 4 * g + 2 * pr:4 * g + 2 * pr + 2, :],
                                        start=False, stop=True, skip_group_check=True))
                                    if i == 5:
                                        chain(box["mms"])
                                        nc.scalar.activation(
                                            et[:, lkc, ts(g, 4), :], pL, AF.Exp)
                                        if g == 1:
                                            nc.vector.tensor_tensor(
                                                out=tt[:, lkc, :, :], in0=et[:, lkc, :, :],
                                                in1=enb[:, lkc, :, :], op=ALU.mult)
                            return f

                        for i in (0, 1, 2, 3, 4, 5):
                            cls.append(mk(lkc, g, i, box))
                return cls, tt

            def make_att_closures(st, rr, tt):
                """Single-mm closures for EFS/merge/tp/out of row s, ordered so
                each PSUM wt-ring bank alternates A,B and every PE consumer of a
                DVE result has big matmuls interleaved before it."""
                sp = st["sp"]
                s = 2 * sp + rr
                tv, sv = st["tv_r"][rr], st["sv_r"][rr]
                mt = actp.tile([128, 2, HC], bf16, tag="mt", name="mt")
                ob = actp.tile([128, 2, OD], bf16, tag="ob", name="ob")
                small = []
                state = {}

                def mk_efs(lqc, h, lkc):
                    def f():
                        if h == 0 and lkc == 0:
                            state[("wa", lqc)] = dict(
                                t=psp.tile([128, 512], f32, tag="wt", bufs=2,
                                           name="wt"), mms=[])
                        box = state[("wa", lqc)]
                        waT = box["t"][:, 0:264].rearrange("p (h x) -> p h x", x=33)
                        box["mms"].append(nc.tensor.matmul(
                            waT[:, h, :], tt[:, lkc, h, ts(lqc, 128)],
                            sv[:, lkc, h, :],
                            start=(h == 0 and lkc == 0), stop=(h == 7 and lkc == 1),
                            skip_group_check=True))
                        if h == 7 and lkc == 1:
                            chain(box["mms"])
                            rs = actp.tile([128, H], f32, tag="rs", bufs=2, name="rs")
                            nc.vector.reciprocal_approx_fast(out=rs, in_=waT[:, :, 32])
                            tmp = actp.tile([128, H, 32], bf16, tag="tmp", bufs=2,
                                            name="tmp")
                            nc.vector.scalar_tensor_tensor(
                                out=tmp,
                                in0=tv[:, lqc, :].rearrange("p (h c) -> p h c", h=H),
                                scalar=1.0, in1=waT[:, :, 0:32],
                                op0=ALU.add, op1=ALU.mult)
                            nc.vector.scalar_tensor_tensor(
                                out=mt[:, lqc, :].rearrange("p (h c) -> p h c", h=H),
                                in0=tmp, scalar=1.0,
                                in1=rs[:, :].unsqueeze(2).broadcast_to([128, H, 32]),
                                op0=ALU.mult, op1=ALU.mult)
                    return f

                def mk_tp(lqc, hcc):
                    def f():
                        if hcc == 0:
                            state[("tp", lqc)] = dict(
                                t=psp.tile([128, 512], f32, tag="wt", bufs=2,
                                           name="wt"), mms=[])
                        box = state[("tp", lqc)]
                        mtT = box["t"][:, 0:128].bitcast(bf16).rearrange(
                            "p (a l) -> p a l", a=2)
                        box["mms"].append(nc.tensor.matmul(
                            mtT[:, hcc, :], mt[:, lqc, ts(hcc, 128)], ident,
                            is_transpose=True, start=(hcc == 0), stop=(hcc == 1),
                            skip_group_check=True))
                        if hcc == 1:
                            chain(box["mms"])
                            mts = actp.tile([128, 2, 128], bf16, tag="mts", bufs=2,
                                            name="mts")
                            nc.vector.tensor_copy(mts, mtT)
                            state[("mts", lqc)] = mts
                    return f

                def mk_po(lqc, k):
                    def f():
                        if k == 0:
                            state[("po", lqc)] = dict(
                                t=psp.tile([128, 512], f32, tag="wt", bufs=2,
                                           name="wt"), mms=[])
                        box = state[("po", lqc)]
                        po = box["t"][:, 0:256]
                        mts = state[("mts", lqc)]
                        if k < 2:
                            box["mms"].append(nc.tensor.matmul(
                                po, mts[:, k, :], wo[:, k, :],
                                start=(k == 0), stop=False, skip_group_check=True))
                        else:
                            box["mms"].append(nc.tensor.matmul(
                                po, onesr, bor,
                                start=False, stop=True, skip_group_check=True))
                            chain(box["mms"])
                            nc.scalar.copy(ob[:, lqc, :], po)
                            nc.sync.dma_start(
                                out=out_d[s, lqc].rearrange("p o -> p o"),
                                in_=ob[:, lqc, :])
                    return f

                # interleave lqc0/lqc1 EFS mms so consecutive matmuls hit
                # alternating PSUM banks (hides same-bank turnaround)
                for h in range(H):
                    for lkc in range(2):
                        for lqc in range(2):
                            small.append(mk_efs(lqc, h, lkc))
                for lqc in range(2):
                    for hcc in range(2):
                        small.append(mk_tp(lqc, hcc))
                    for k in range(3):
                        small.append(mk_po(lqc, k))
                return small

            prev = None
            for sp in range(RPC // 2):
                pf = emit_pair_front(sp)
                st, pair_cls = make_pair_closures(sp, pf)
                for f in pair_cls:
                    f()
                st["bias"] = [emit_front(2 * sp), emit_front(2 * sp + 1)]
                if prev is not None:
                    for rr in range(2):
                        logs, tt = make_log_closures(prev, rr, prev["bias"][rr])
                        for f in logs:
                            f()
                        for f in make_att_closures(prev, rr, tt):
                            f()
                prev = st
            for rr in range(2):
                logs, tt = make_log_closures(prev, rr, prev["bias"][rr])
                for f in logs:
                    f()
                for f in make_att_closures(prev, rr, tt):
                    f()

    nc.compile()
    return nc


def _prep_inputs(q_data, kv_data, bias, nonbatched_bias, Wq, Wk, Wv, Wg, bg, Wo, bo):
    """Host-side data marshalling only (layout + dtype cast; qscale folded
    into Wq as a constant rescale)."""
    import ml_dtypes
    c = np.ascontiguousarray
    bf = ml_dtypes.bfloat16
    f = np.float32

    def pair_layout(x):  # x [b2, l, d] -> [b2/2, 2(c2), 128, 2(row), l]
        xt = np.swapaxes(np.asarray(x, f), 1, 2)          # [b2, d, l]
        xt = xt.reshape(B2 // 2, 2, 2, 128, xt.shape[-1])  # [sp, row, c2, p, l]
        return c(np.transpose(xt, (0, 2, 3, 1, 4))).astype(bf)

    qdT = pair_layout(q_data[0])
    kvdT = pair_layout(kv_data[0])
    biasT = c(np.transpose(np.asarray(bias[0], f), (0, 3, 1, 2))).reshape(B2, 2, 128, H, LQ).astype(bf)
    nbT = c(np.transpose(np.asarray(nonbatched_bias[0], f), (2, 0, 1))).reshape(2, 128, H, LQ).astype(bf)
    qscale = float(C) ** -0.5
    wqT = c((np.asarray(Wq, f) * qscale).T).astype(bf)
    wkT = c(np.asarray(Wk, f).T).astype(bf)
    wvT = c(np.asarray(Wv, f).T).astype(bf)
    wgT = c(np.asarray(Wg, f).T).astype(bf)
    woT = c(np.asarray(Wo, f).T).astype(bf)
    ident = np.eye(128, dtype=f).astype(bf)
    onesr = np.ones((1, 128), f).astype(bf)
    bgr = np.asarray(bg, f)[None, :].astype(bf)
    bor = np.asarray(bo, f)[None, :].astype(bf)

    in_maps = []
    for core in range(NCORES):
        sl = slice(core * RPC, (core + 1) * RPC)
        in_maps.append(dict(
            qdT=c(qdT[core * RPC // 2:(core + 1) * RPC // 2]),
            kvdT=c(kvdT[core * RPC // 2:(core + 1) * RPC // 2]),
            biasT=c(biasT[sl]), nbT=nbT,
            wqT=wqT, wkT=wkT, wvT=wvT, wgT=wgT, woT=woT,
            ident=ident, onesr=onesr, bgr=bgr, bor=bor,
        ))
    return in_maps


def kernel(q_data, kv_data, bias, nonbatched_bias, Wq, Wk, Wv, Wg, bg, Wo, bo,
           _trace=False):
    from concourse.bass_utils import run_bass_kernel_spmd

    if "nc" not in _CACHE:
        _CACHE["nc"] = _build_nc()
    nc = _CACHE["nc"]
    in_maps = _prep_inputs(q_data, kv_data, bias, nonbatched_bias,
                           Wq, Wk, Wv, Wg, bg, Wo, bo)
    res = run_bass_kernel_spmd(nc, in_maps, list(range(NCORES)), trace=_trace)
    # out shard [RPC, 2, 128, OD] bf16 -> [RPC, LQ, OD] f32
    out = np.concatenate(
        [np.asarray(res.results[i]["out"]).astype(np.float32).reshape(RPC, LQ, OD)
         for i in range(NCORES)], axis=0)
    out = out.reshape(B1, B2, LQ, OD)
    if _trace:
        _CACHE["last_result"] = res
    return out
